# revision 2
# baseline (speedup 1.0000x reference)
"""Trainium2 Bass kernel v2 for DynamicTaskMemoryInduction.

The axon-tunneled trn2 terminal costs ~36us PER DISPATCHED INSTRUCTION
(engine- and size-independent; matmul calls cost 2 units, DMAs ~2.5).
So v2 minimizes instruction count, not data movement:
  - bf16 hi-only matmuls (no split): rel err ~5e-3, gate is 2e-2.
  - transposes via DRAM bounce (4 writes + 2 gather reads) instead of PE.
  - coeff^T via XBAR dma_start_transpose (2 DMAs/iter).
  - num+G fused in one matmul set (lhsT = [uT|tmT]).
  - packed elementwise: QP=[pag|sm1|num] -> one mul+reduce for 3 qforms;
    AG=[agree|mv*sm1] in one broadcast mul.
  - tanh/rsqrt via Ln/Exp only (single act table load).

Math identities as in the baseline (see git history): pearson num via
centered u only; recursive num/ssq updates with lam=2^i scaling; agree
via the constant Gram matrix G_n = tm_n @ tm_n^T.

Sharding: data-parallel over Q across 8 cores (64 queries/core).
"""

import numpy as np

EPS = 1e-8
Q, I, C, N, D = 512, 768, 64, 4, 192
ND, NC = N * D, N * C
NCORES = 8
QL = Q // NCORES


def build(reps=1, stop_at=None):
    import concourse.bacc as bacc
    import concourse.tile as tile
    import concourse.mybir as mybir

    F32 = mybir.dt.float32
    BF16 = mybir.dt.bfloat16
    AF = mybir.ActivationFunctionType
    OP = mybir.AluOpType
    AX = mybir.AxisListType

    nc = bacc.Bacc("TRN2", target_bir_lowering=False, debug=False,
                   num_devices=NCORES)

    wsqm_d = nc.dram_tensor("wsqm", [I, 896], BF16, kind="ExternalInput").ap()
    b_d = nc.dram_tensor("b_r", [C, ND], F32, kind="ExternalInput").ap()
    out_d = nc.dram_tensor("out", [QL, ND], F32, kind="ExternalOutput").ap()

    KC = I // 128  # 6 contraction chunks

    with tile.TileContext(nc) as tc:
        with tc.tile_pool(name="const", bufs=1) as cp, \
             tc.tile_pool(name="big", bufs=1) as bp, \
             tc.tile_pool(name="sm", bufs=1) as sp:

            eps_t = cp.tile([QL, 1], F32, tag="eps")
            nc.gpsimd.memset(eps_t[:], EPS)

            def _dbg_out(src_ap, width):
                dbg = bp.tile([QL, ND], F32, tag="dbg")
                nc.gpsimd.memset(dbg[:], 0.0)
                nc.vector.tensor_copy(dbg[:, 0:width], src_ap)
                nc.sync.dma_start(out_d[:], dbg[:])

            for rep in range(reps):
                bnc = nc.dram_tensor(f"bnc{rep}", [ND, 128], BF16).ap()
                st_dr = nc.dram_tensor(f"st{rep}", [C, 8], F32).ap()

                # ---- load inputs (2 DMAs) ----
                wq = cp.tile([128, KC * 896], BF16, tag="wq")
                nc.sync.dma_start(
                    out=wq[:].rearrange("p (k f) -> p k f", k=KC),
                    in_=wsqm_d[:].rearrange("(k p) f -> p k f", p=128))
                b_sb = cp.tile([C, ND], F32, tag="b")
                nc.gpsimd.dma_start(b_sb[:], b_d[:])
                if stop_at == "load":
                    _dbg_out(b_sb[:], ND)
                    continue

                # ---- phase A: [tq; tm] = qm^T @ wsT (12 matmuls) ----
                with tc.tile_pool(name="psA", bufs=1, space="PSUM") as psA:
                    ps_a = psA.tile([128, ND], F32, tag="a")
                    for k in range(KC):
                        lhsT = wq[:, k * 896 + 768: k * 896 + 896]
                        for c0, c1 in ((0, 512), (512, 768)):
                            nc.tensor.matmul(ps_a[:, c0:c1], lhsT,
                                             wq[:, k * 896 + c0: k * 896 + c1],
                                             start=(k == 0), stop=(k == KC - 1))

                    # tmu rows 0:64 = u (centered tq), 64:128 = tm, bf16
                    tmu = bp.tile([128, ND], BF16, tag="tmu")
                    nc.vector.tensor_add(tmu[64:128, :], ps_a[64:128, :], b_sb[:])
                    s1q = sp.tile([QL, N], F32, tag="s1q")
                    nc.vector.tensor_reduce(
                        out=s1q[:],
                        in_=ps_a[0:64, :].rearrange("p (n d) -> p n d", n=N),
                        axis=AX.X, op=OP.add)
                    nc.vector.scalar_tensor_tensor(
                        out=tmu[0:64, :].rearrange("p (n d) -> p n d", n=N),
                        in0=s1q[:].unsqueeze(2).broadcast_to([QL, N, D]),
                        scalar=-1.0 / D,
                        in1=ps_a[0:64, :].rearrange("p (n d) -> p n d", n=N),
                        op0=OP.mult, op1=OP.add)

                if stop_at == "phaseA":
                    _dbg_out(tmu[0:64, :], ND)
                    continue
                u_t = tmu[0:64, :]
                tm_t = tmu[64:128, :]

                # ssq0 = sum_d u^2 per n
                usq = bp.tile([QL, ND], F32, tag="usq")
                nc.vector.tensor_mul(usq[:], u_t, u_t)
                ssq = sp.tile([QL, N], F32, tag="ssq0")
                nc.vector.tensor_reduce(
                    out=ssq[:], in_=usq[:].rearrange("p (n d) -> p n d", n=N),
                    axis=AX.X, op=OP.add)

                # tm stats -> ST [64, (x n)]: x=0 ssm, x=1 s1m
                ST = sp.tile([C, 8], F32, tag="ST")
                nc.vector.tensor_reduce(
                    out=ST[:, 4:8], in_=tm_t.rearrange("p (n d) -> p n d", n=N),
                    axis=AX.X, op=OP.add)
                tmsq = bp.tile([C, ND], F32, tag="tmsq")
                nc.vector.tensor_mul(tmsq[:], tm_t, tm_t)
                sq2m = sp.tile([C, N], F32, tag="sq2m")
                nc.vector.tensor_reduce(
                    out=sq2m[:], in_=tmsq[:].rearrange("p (n d) -> p n d", n=N),
                    axis=AX.X, op=OP.add)
                s1m2 = sp.tile([C, N], F32, tag="s1m2")
                nc.vector.tensor_mul(s1m2[:], ST[:, 4:8], ST[:, 4:8])
                nc.vector.scalar_tensor_tensor(
                    out=ST[:, 0:4], in0=s1m2[:], scalar=-1.0 / D, in1=sq2m[:],
                    op0=OP.mult, op1=OP.add)

                # stats bounce: [64c,8] -> DRAM -> [1,512] -> broadcast [64,512]
                # STb[q, c*8 + x*4 + n]
                nc.sync.dma_start(st_dr[:], ST[:])
                row = sp.tile([1, 512], F32, tag="row")
                nc.sync.dma_start(
                    row[:], st_dr[:].rearrange("c k -> (c k)").unsqueeze(0))
                STb = bp.tile([QL, 512], F32, tag="STb")
                nc.gpsimd.partition_broadcast(STb[:], row[:])
                if stop_at == "stats":
                    _dbg_out(STb[:], 512)
                    continue
                ssm_v = STb[:].rearrange("p (c x n) -> p x n c", x=2, n=N)[:, 0]
                sm1_v = STb[:].rearrange("p (c x n) -> p x n c", x=2, n=N)[:, 1]

                # tmu bounce: bnc[n*D+d, xj] = tmu[xj, n*D+d]
                for n in range(N):
                    nc.sync.dma_start(
                        out=bnc[n * D:(n + 1) * D, :].rearrange("d p -> p d"),
                        in_=tmu[:, n * D:(n + 1) * D])
                # trA[d, n*128 + xj]: cols n*128+(0:64)=uT_n, +(64:128)=tmT_n
                trA = bp.tile([128, 512], BF16, tag="trA")
                nc.sync.dma_start(
                    out=trA[:].rearrange("d (n p) -> d n p", n=N),
                    in_=bnc[:].rearrange("(n d) p -> d n p", n=N)[0:128])
                trB = bp.tile([64, 512], BF16, tag="trB")
                nc.sync.dma_start(
                    out=trB[:].rearrange("d (n p) -> d n p", n=N),
                    in_=bnc[:].rearrange("(n d) p -> d n p", n=N)[128:192])

                if stop_at == "bounce":
                    trAf = bp.tile([128, 512], F32, tag="trAf")
                    nc.vector.tensor_copy(trAf[:], trA[:])
                    _dbg_out(trAf[0:64, :], 512)
                    continue
                # QP ping-pong tiles: [pag | sm1 | num]
                QP0 = bp.tile([QL, 768], F32, tag="QP0")
                QP1 = bp.tile([QL, 768], F32, tag="QP1")
                QP = [QP0, QP1]
                for i in (0, 1):
                    nc.vector.tensor_copy(
                        QP[i][:, 256:512].rearrange("p (n c) -> p n c", n=N),
                        sm1_v)

                with tc.tile_pool(name="ps2", bufs=1, space="PSUM") as ps2:
                    # numG: rows 0:64 num [q,c], rows 64:128 G [c',c]
                    numG = ps2.tile([128, NC], F32, tag="numG")
                    for n in range(N):
                        sl = (slice(None), slice(n * C, (n + 1) * C))
                        nc.tensor.matmul(
                            numG[sl], trA[:, n * 128:(n + 1) * 128],
                            trA[:, n * 128 + 64: n * 128 + 128],
                            start=True, stop=False)
                        nc.tensor.matmul(
                            numG[sl], trB[:, n * 128:(n + 1) * 128],
                            trB[:, n * 128 + 64: n * 128 + 128],
                            start=False, stop=True)
                    nc.vector.tensor_copy(QP[0][:, 512:768], numG[0:64, :])
                    # block-diagonal G per n-pair: pag for 2 n's in ONE
                    # 128-contraction matmul, no PE array tiling involved
                    BD = bp.tile([128, NC], BF16, tag="BD")
                    nc.gpsimd.memset(BD[:], 0.0)
                    nc.vector.tensor_copy(BD[0:64, 0:64], numG[64:128, 0:64])
                    nc.vector.tensor_copy(BD[64:128, 64:128],
                                          numG[64:128, 64:128])
                    nc.vector.tensor_copy(BD[0:64, 128:192],
                                          numG[64:128, 128:192])
                    nc.vector.tensor_copy(BD[64:128, 192:256],
                                          numG[64:128, 192:256])
                # block-diagonal tm per n-pair for the final hv matmuls
                tmBD = bp.tile([128, ND], BF16, tag="tmBD")
                nc.gpsimd.memset(tmBD[:], 0.0)
                nc.vector.tensor_copy(tmBD[0:64, 0:192], tm_t[:, 0:192])
                nc.vector.tensor_copy(tmBD[64:128, 192:384], tm_t[:, 192:384])
                nc.vector.tensor_copy(tmBD[0:64, 384:576], tm_t[:, 384:576])
                nc.vector.tensor_copy(tmBD[64:128, 576:768], tm_t[:, 576:768])

                if stop_at == "numG":
                    _dbg_out(QP[0][:, 0:768], 768)
                    continue

                def make_p(num_ap, ssq_t, tag):
                    """p = tanh(-num * rsqrt(ssm*ssq)); Ln/Exp only."""
                    den2 = bp.tile([QL, NC], F32, tag="den2")
                    nc.vector.tensor_mul(
                        den2[:].rearrange("p (n c) -> p n c", n=N),
                        ssm_v,
                        ssq_t[:].unsqueeze(2).broadcast_to([QL, N, C]))
                    lnd = bp.tile([QL, NC], F32, tag="lnd")
                    nc.scalar.activation(lnd[:], den2[:], AF.Ln)
                    rsq = bp.tile([QL, NC], F32, tag="rsq")
                    nc.scalar.activation(rsq[:], lnd[:], AF.Exp, scale=-0.5)
                    r_t = bp.tile([QL, NC], F32, tag="r")
                    nc.vector.tensor_mul(r_t[:], num_ap, rsq[:])
                    e2 = bp.tile([QL, NC], F32, tag="e2")
                    nc.scalar.activation(e2[:], r_t[:], AF.Exp, scale=-2.0)
                    t1 = bp.tile([QL, NC], F32, tag="t1")
                    nc.vector.tensor_scalar_add(t1[:], e2[:], 1.0)
                    t1r = bp.tile([QL, NC], F32, tag="t1r")
                    nc.vector.reciprocal(t1r[:], t1[:])
                    p_t = bp.tile([QL, NC], F32, tag=tag)
                    nc.vector.tensor_scalar(out=p_t[:], in0=t1r[:], scalar1=-2.0,
                                            scalar2=1.0, op0=OP.mult, op1=OP.add)
                    return p_t

                p_t = make_p(QP[0][:, 512:768], ssq, "p")
                if stop_at == "p0":
                    _dbg_out(p_t[:], NC)
                    continue
                a_t = None

                def softmax_coeff(a_ap, p_ap):
                    e_t = bp.tile([QL, NC], F32, tag="e")
                    nc.scalar.activation(e_t[:], a_ap, AF.Exp)
                    rs = sp.tile([QL, C], F32, tag="rs")
                    nc.vector.tensor_reduce(
                        out=rs[:], in_=e_t[:].rearrange("p (n c) -> p c n", n=N),
                        axis=AX.X, op=OP.add)
                    rsi = sp.tile([QL, C], F32, tag="rsi")
                    nc.vector.reciprocal(rsi[:], rs[:])
                    dd = bp.tile([QL, NC], F32, tag="dd")
                    nc.vector.tensor_mul(
                        dd[:].rearrange("p (n c) -> p n c", n=N),
                        e_t[:].rearrange("p (n c) -> p n c", n=N),
                        rsi[:].unsqueeze(1).broadcast_to([QL, N, C]))
                    cf = bp.tile([QL, NC], BF16, tag="cf")
                    nc.vector.tensor_add(cf[:], dd[:], p_ap)
                    return cf

                def transpose_cf(cf):
                    cT = bp.tile([128, 128], BF16, tag="cT")
                    nc.sync.dma_start_transpose(cT[:, 0:64], cf[:, 0:128])
                    nc.sync.dma_start_transpose(cT[:, 64:128], cf[:, 128:256])
                    return cT

                with tc.tile_pool(name="psI", bufs=1, space="PSUM") as psI:
                    lam = 1.0
                    for it in (1, 2):
                        qp_in = QP[(it - 1) % 2]
                        qp_out = QP[it % 2]
                        if it == 1:
                            cf = bp.tile([QL, NC], BF16, tag="cf")
                            nc.vector.tensor_scalar_add(cf[:], p_t[:], 1.0 / N)
                        else:
                            cf = softmax_coeff(a_t[:], p_t[:])
                        if stop_at == f"cf{it}":
                            _dbg_out(cf[:], NC)
                            break
                        cT = transpose_cf(cf)
                        if stop_at == f"cT{it}":
                            cTf = bp.tile([128, 128], F32, tag="cTf")
                            nc.vector.tensor_copy(cTf[:], cT[:])
                            _dbg_out(cTf[0:64, :], 128)
                            break

                        pag = psI.tile([QL, NC], F32, tag="pag")
                        nc.tensor.matmul(pag[:, 0:128], cT[:, 0:64],
                                         BD[:, 0:128], start=True, stop=True)
                        nc.tensor.matmul(pag[:, 128:256], cT[:, 64:128],
                                         BD[:, 128:256], start=True, stop=True)
                        nc.vector.tensor_copy(qp_in[:, 0:256], pag[:])
                        if stop_at == f"pag{it}":
                            _dbg_out(qp_in[:, 0:256], NC)
                            break

                        # qforms: PR = cf (x3 bcast) * [pag|sm1|num]
                        PR = bp.tile([QL, 768], F32, tag="PR")
                        nc.vector.tensor_mul(
                            PR[:].rearrange("p (x n c) -> p x n c", x=3, n=N),
                            cf[:].rearrange("p (n c) -> p n c", n=N)
                                 .unsqueeze(1).broadcast_to([QL, 3, N, C]),
                            qp_in[:].rearrange("p (x n c) -> p x n c", x=3, n=N))
                        if stop_at == f"PR{it}":
                            _dbg_out(PR[:], 768)
                            break
                        Q3 = sp.tile([QL, 12], F32, tag="Q3")
                        nc.vector.tensor_reduce(
                            out=Q3[:],
                            in_=PR[:].rearrange("p (x n c) -> p x n c", x=3, n=N),
                            axis=AX.X, op=OP.add)
                        sshv, s1hv, qf1 = Q3[:, 0:4], Q3[:, 4:8], Q3[:, 8:12]
                        if stop_at == f"Q3{it}":
                            _dbg_out(Q3[:], 12)
                            break

                        # s = recip((sshv+1) * rsqrt(sshv)); mv = s1hv/D * s
                        lns = sp.tile([QL, N], F32, tag="lns")
                        nc.scalar.activation(lns[:], sshv, AF.Ln)
                        rss = sp.tile([QL, N], F32, tag="rss")
                        nc.scalar.activation(rss[:], lns[:], AF.Exp, scale=-0.5)
                        sinv = sp.tile([QL, N], F32, tag="sinv")
                        nc.vector.scalar_tensor_tensor(
                            out=sinv[:], in0=sshv, scalar=1.0, in1=rss[:],
                            op0=OP.add, op1=OP.mult)
                        W = sp.tile([QL, 16], F32, tag="W")
                        s_t, mv_t = W[:, 0:4], W[:, 4:8]
                        nc.vector.reciprocal(s_t, sinv[:])
                        nc.vector.scalar_tensor_tensor(
                            out=mv_t, in0=s1hv, scalar=1.0 / D, in1=s_t,
                            op0=OP.mult, op1=OP.mult)
                        # W[8:16] = [s^2 | mv^2]
                        nc.vector.tensor_mul(W[:, 8:16], W[:, 0:8], W[:, 0:8])

                        # AG = [s|mv] (bcast c) * [pag|sm1] -> [agree | q1]
                        AGt = bp.tile([QL, 512], F32, tag="AG")
                        nc.vector.tensor_mul(
                            AGt[:].rearrange("p (x n c) -> p x n c", x=2, n=N),
                            W[:, 0:8].rearrange("p (x n) -> p x n", x=2)
                                     .unsqueeze(3).broadcast_to([QL, 2, N, C]),
                            qp_in[:, 0:512].rearrange("p (x n c) -> p x n c",
                                                      x=2, n=N))
                        if stop_at == f"AG{it}":
                            _dbg_out(AGt[:], 512)
                            break
                        agree = AGt[:, 0:256]
                        q2 = bp.tile([QL, NC], F32, tag="q2")
                        nc.vector.tensor_sub(q2[:], agree, AGt[:, 256:512])
                        nc.vector.scalar_tensor_tensor(
                            out=qp_out[:, 512:768], in0=q2[:], scalar=lam,
                            in1=qp_in[:, 512:768], op0=OP.mult, op1=OP.add)

                        # ssq' = ssq + 2lam*s*qf1 + lam^2*(s^2*sshv - D*mv^2)
                        z1 = sp.tile([QL, N], F32, tag="z1")
                        nc.vector.tensor_mul(z1[:], W[:, 8:12], sshv)
                        z2 = sp.tile([QL, N], F32, tag="z2")
                        nc.vector.tensor_mul(z2[:], s_t, qf1)
                        u1 = sp.tile([QL, N], F32, tag="u1")
                        nc.vector.scalar_tensor_tensor(
                            out=u1[:], in0=z2[:], scalar=2.0 * lam, in1=ssq[:],
                            op0=OP.mult, op1=OP.add)
                        u2 = sp.tile([QL, N], F32, tag="u2")
                        nc.vector.scalar_tensor_tensor(
                            out=u2[:], in0=z1[:], scalar=lam * lam, in1=u1[:],
                            op0=OP.mult, op1=OP.add)
                        ssq_n = sp.tile([QL, N], F32, tag=f"ssq{it}")
                        nc.vector.scalar_tensor_tensor(
                            out=ssq_n[:], in0=W[:, 12:16],
                            scalar=-float(D) * lam * lam, in1=u2[:],
                            op0=OP.mult, op1=OP.add)
                        ssq = ssq_n
                        if stop_at == f"ssq{it}":
                            _dbg_out(ssq[:], N)
                            break

                        # a update
                        if it == 1:
                            a_t = bp.tile([QL, NC], F32, tag="a")
                            nc.vector.tensor_mul(a_t[:], p_t[:], agree)
                        else:
                            pa = bp.tile([QL, NC], F32, tag="pa")
                            nc.vector.tensor_mul(pa[:], p_t[:], agree)
                            a_n = bp.tile([QL, NC], F32, tag="a2")
                            nc.vector.tensor_add(a_n[:], a_t[:], pa[:])
                            a_t = a_n

                        lam *= 2.0
                        p_t = make_p(qp_out[:, 512:768], ssq, "p")
                        if stop_at == f"it{it}":
                            break

                    if stop_at is not None and stop_at != "full":
                        if stop_at in ("it1", "it2"):
                            _dbg_out(a_t[:], NC)
                        continue
                    # ---- final ----
                    cf = softmax_coeff(a_t[:], p_t[:])
                    cT = transpose_cf(cf)
                    # n-pair outputs at 512-f32 strides: each 384-col block
                    # stays inside one PSUM bank
                    hv = psI.tile([QL, 1024], F32, tag="hv")
                    nc.tensor.matmul(hv[:, 0:384], cT[:, 0:64],
                                     tmBD[:, 0:384], start=True, stop=True)
                    nc.tensor.matmul(hv[:, 512:896], cT[:, 64:128],
                                     tmBD[:, 384:768], start=True, stop=True)
                    hv_v = (hv[:].rearrange("p (g e) -> p g e", g=2)
                            [:, :, 0:384]
                            .rearrange("p g (j d) -> p g j d", j=2))
                    hv_sb = bp.tile([QL, ND], F32, tag="hv_sb")
                    nc.vector.tensor_copy(
                        hv_sb[:].rearrange("p (g j d) -> p g j d", g=2, j=2),
                        hv_v)
                    hsq = bp.tile([QL, ND], F32, tag="hsq")
                    nc.vector.tensor_mul(hsq[:], hv_sb[:], hv_sb[:])
                    sf = sp.tile([QL, N], F32, tag="sf")
                    nc.vector.tensor_reduce(
                        out=sf[:], in_=hsq[:].rearrange("p (n d) -> p n d", n=N),
                        axis=AX.X, op=OP.add)
                    lnf = sp.tile([QL, N], F32, tag="lnf")
                    nc.scalar.activation(lnf[:], sf[:], AF.Ln, bias=eps_t[:])
                    rsf = sp.tile([QL, N], F32, tag="rsf")
                    nc.scalar.activation(rsf[:], lnf[:], AF.Exp, scale=-0.5)
                    sinvf = sp.tile([QL, N], F32, tag="sinvf")
                    nc.vector.scalar_tensor_tensor(
                        out=sinvf[:], in0=sf[:], scalar=1.0, in1=rsf[:],
                        op0=OP.add, op1=OP.mult)
                    scf = sp.tile([QL, N], F32, tag="scf")
                    nc.vector.reciprocal(scf[:], sinvf[:])
                    out_sb = bp.tile([QL, ND], F32, tag="out")
                    nc.vector.tensor_mul(
                        out_sb[:].rearrange("p (n d) -> p n d", n=N),
                        hv_sb[:].rearrange("p (n d) -> p n d", n=N),
                        scf[:].unsqueeze(2).broadcast_to([QL, N, D]))
                    nc.sync.dma_start(out_d[:], out_sb[:])

    # single combined act table (Ln/Exp) — see baseline comment
    import concourse.bacc as bacc_mod
    from concourse.hw_specs import get_activation_tables as _real_gat

    def _gat_combined_only(arch):
        tables = _real_gat(arch)
        return {name: (funcs if name == "natural_log_exp_and_others" else set())
                for name, funcs in tables.items()}

    bacc_mod.get_activation_tables = _gat_combined_only
    try:
        nc.compile()
    finally:
        bacc_mod.get_activation_tables = _real_gat
    return nc
